# revision 1
# baseline (speedup 1.0000x reference)
"""Distributed Trainium2 kernel for the per-agent trajectory attention module.

Math (per reference):
    q = received_messages @ Wq + bq                    [512, 512]
    k = taus @ Wk + bk ; v = taus @ Wv + bv            [16*512, 512/64]
    scores[i, t] = dot(q[i], k[t, i]) / sqrt(512)
    messages[i] = sum_t softmax(scores)[i, t] * v[t, i]  [512, 64]

Sharding over 8 NeuronCores:
  - q: tensor-parallel over the 32768 msg dim (4096 per core), partial q for
    ALL 512 agents per core, one f32 ReduceScatter(add) over the agent axis ->
    each core holds q for its own 64 agents.  The 1/sqrt(H) scale is folded
    into Wq on the host and bq*scale/8 into each core's pre-collective drain,
    so nothing downstream of the collective needs a fixup op.  bk drops out
    of softmax exactly (per-agent constant shift); bv is added at the end.
  - k/v/attention: data-parallel over agents (64 per core).
  - matmul operands stored/computed in bf16 (halves the DMA roofline);
    PSUM accumulation and everything after the matmuls is f32.

Self-contained: hardcodes all shapes; host-side packs inputs into
block-major, contraction-dim-on-partitions layouts so every DMA is fully
contiguous and no on-device transposes are needed anywhere.  Scores use a
fused multiply+row-accumulate on VectorE against SBUF-parked k; softmax runs
on ScalarE (Exp with fused -max bias and row-sum accumulator); the weighted
v-sum uses per-partition-scalar ops, with the odd-step partition half merged
into the output by a CCE accumulate-DMA.
"""

import math

import numpy as np

T = 16
N_AGENTS = 512
TAU = 2048
MSG = 32768
HID = 512
DV = 64

NC = 8
AG = N_AGENTS // NC  # 64 agents per core
GS = 8  # group size: the q reduce-scatter spans all 8 cores
NG = NC // GS
GAG = N_AGENTS // NG  # agents per group (512)
MS = MSG // GS  # 4096 msg columns per core
KQ = MS // 128  # 32 contraction chunks for q
KT = TAU // 128  # 16 contraction chunks for k/v
RT = (T * AG) // 128  # 8 row-tiles of taus per core (128 rows each)

SCALE = 1.0 / math.sqrt(HID)

# compute/storage dtype for the big matmul operands: "f32r" (full f32 storage,
# tf32-ish matmul precision) or "bf16" (half the DMA bytes, bf16 matmuls)
DTYPE = "bf16"
WARMUP_MMS = 48  # dummy matmuls to lift the PE HAM throttle before real work
SCORES_STT = True  # fused mul+accum scores via scalar_tensor_tensor

_CACHE = {}

# set by test harness: run with trace and stash exec time here
TRACE = False
LAST_EXEC_NS = None
LAST_RESULTS = None


def _build(stage="full"):
    import concourse.bacc as bacc
    import concourse.mybir as mybir
    import concourse.tile as tile
    from concourse.tile import add_dep_helper

    f32 = mybir.dt.float32
    f32r = mybir.dt.float32r if DTYPE == "f32r" else mybir.dt.bfloat16
    add = mybir.AluOpType.add
    mult = mybir.AluOpType.mult

    nc = bacc.Bacc("TRN2", target_bir_lowering=False, debug=False, num_devices=NC)

    # inputs (per-core shards, pre-packed host-side; layout [128, kc, n])
    rm_d = nc.dram_tensor("rm", [GAG // 128, 128, KQ, 128], f32r, kind="ExternalInput")
    wq_d = nc.dram_tensor("wq", [128, KQ, HID], f32r, kind="ExternalInput")
    traj_d = nc.dram_tensor("traj", [RT, 128, KT, 128], f32r, kind="ExternalInput")
    wk_d = nc.dram_tensor("wk", [128, KT, HID], f32r, kind="ExternalInput")
    wv_d = nc.dram_tensor("wv", [128, KT, DV], f32r, kind="ExternalInput")
    bqs_d = nc.dram_tensor("bqs", [128, HID], f32, kind="ExternalInput")  # bq * SCALE
    bv_d = nc.dram_tensor("bv", [AG, DV], f32, kind="ExternalInput")
    out_d = nc.dram_tensor("out", [AG, DV], f32, kind="ExternalOutput")

    cdt = f32  # CCE bf16 reduce is broken on HW; keep the collective f32
    rs_in = nc.dram_tensor("rs_in", [GAG, HID], cdt)
    rs_out = nc.dram_tensor("rs_out", [AG, HID], cdt)


    with tile.TileContext(nc) as tc:
        with (
            tc.tile_pool(name="res", bufs=1) as res,
            tc.tile_pool(name="rmp", bufs=4) as rmp,
            tc.tile_pool(name="tjp", bufs=3) as tjp,
            tc.tile_pool(name="work", bufs=2) as work,
            tc.tile_pool(name="qps", bufs=6, space="PSUM") as qps,
            tc.tile_pool(name="vps", bufs=2, space="PSUM") as vps,
        ):
            # ---------------- PE warm-up (HAM unthrottle) ----------------
            if WARMUP_MMS:
                wz = res.tile([128, 128], f32r)
                nc.gpsimd.memset(wz[:], 0.0)
                wacc = qps.tile([128, 512], f32, tag="acc", name="warm_acc")
                for i in range(WARMUP_MMS):
                    nc.tensor.matmul(
                        wacc[:, 0:128],
                        wz[:],
                        wz[:],
                        start=(i == 0),
                        stop=(i == WARMUP_MMS - 1),
                    )

            # ---------------- resident tensors ----------------
            bqs_sb = res.tile([128, HID], f32)
            bv_sb = res.tile([AG, DV], f32)
            nc.scalar.dma_start(bqs_sb[:], bqs_d[:])
            nc.scalar.dma_start(bv_sb[:], bv_d[:])

            wq_sb = res.tile([128, KQ, HID], f32r)
            rm_tiles = []
            for m in range(GAG // 128):
                rm_sb = rmp.tile([128, KQ, 128], f32r, tag="rm", name=f"rm_sb{m}")
                rm_tiles.append(rm_sb)
            nc.sync.dma_start(rm_tiles[0][:], rm_d[0])
            for w4 in range(8):
                nc.sync.dma_start(
                    wq_sb[:, w4 * (KQ // 8) : (w4 + 1) * (KQ // 8), :],
                    wq_d[:, w4 * (KQ // 8) : (w4 + 1) * (KQ // 8), :],
                )

            for m in range(1, GAG // 128):
                nc.sync.dma_start(rm_tiles[m][:], rm_d[m])

            # ------- q phase: partial q for the group's agents -------
            for m in range(GAG // 128):
                rm_sb = rm_tiles[m]
                qacc = qps.tile([128, HID], f32, tag="acc")
                for kc in range(KQ):
                    nc.tensor.matmul(
                        qacc[:],
                        rm_sb[:, kc, :],
                        wq_sb[:, kc, :],
                        start=(kc == 0),
                        stop=(kc == KQ - 1),
                    )
                qdr = work.tile([128, HID], cdt, tag="qdr")
                # qacc is already scaled (Wq pre-scaled on host); add bq*SCALE/NC
                # here so the ReduceScatter sum carries the bias exactly once
                nc.vector.scalar_tensor_tensor(
                    qdr[:], qacc[:], 1.0, bqs_sb[:], mult, add
                )
                nc.scalar.dma_start(rs_in[m * 128 : (m + 1) * 128, :], qdr[:])

            nc.gpsimd.collective_compute(
                "ReduceScatter",
                add,
                replica_groups=[list(range(NC))],
                ins=[rs_in.ap().opt()],
                outs=[rs_out.ap().opt()],
            )

            # local q, duplicated into both partition halves, scaled + bias
            q2 = res.tile([128, HID], cdt)
            nc.scalar.dma_start(
                q2[:], rs_out.ap().unsqueeze(0).broadcast_to([2, AG, HID])
            )
            q2s = q2

            # ---------------- k/v phase + scores ----------------
            if stage == "q":
                nc.scalar.dma_start(out_d[:], q2[0:AG, 0:DV])
            do_kv = stage in ("kv", "full")
            do_tail = stage == "full"

            wk_sb = res.tile([128, KT, HID], f32r, name="wk_sb") if do_kv else None
            wv_sb = res.tile([128, KT, DV], f32r, name="wv_sb") if do_kv else None
            if do_kv:
                nc.sync.dma_start(wk_sb[:], wk_d[:])
                nc.sync.dma_start(wv_sb[:], wv_d[:])

            v_sb = res.tile([128, RT, DV], f32)
            k_sb = res.tile([128, RT, HID], f32)
            s_scr = res.tile([128, 2 * RT], f32)

            def kv_tile(rt):
                tj_sb = tjp.tile([128, KT, 128], f32r, tag="tj", name=f"tj{rt}")
                nc.sync.dma_start(tj_sb[:], traj_d[rt])
                kacc = qps.tile([128, HID], f32, tag="acc", name=f"kacc{rt}")
                for kc in range(KT):
                    nc.tensor.matmul(
                        kacc[:],
                        tj_sb[:, kc, :],
                        wk_sb[:, kc, :],
                        start=(kc == 0),
                        stop=(kc == KT - 1),
                    )
                vacc = vps.tile([128, DV], f32, tag="vacc", name=f"vacc{rt}")
                for kc in range(KT):
                    nc.tensor.matmul(
                        vacc[:],
                        tj_sb[:, kc, :],
                        wv_sb[:, kc, :],
                        start=(kc == 0),
                        stop=(kc == KT - 1),
                    )
                # park k in SBUF so the PSUM bank frees without waiting on q2
                nc.vector.tensor_copy(k_sb[:, rt, :], kacc[:])
                return nc.vector.tensor_copy(v_sb[:, rt, :], vacc[:])

            def score_tile(rt):
                prod = work.tile([128, HID], f32, tag="ttr", name=f"prod{rt}")
                return nc.vector.scalar_tensor_tensor(
                    prod[:],
                    k_sb[:, rt, :],
                    1.0,
                    q2s[:],
                    mult,
                    mult,
                    accum_out=s_scr[:, rt : rt + 1],
                )

            # PE only needs drains rt0..5 to keep its PSUM slots rotating, so
            # emit scores 0..5 right after drain 5 — they fire the moment the
            # collective lands instead of waiting for the last two drains.
            if do_kv:
                last_drain5 = None
                for rt in range(6):
                    last_drain5 = kv_tile(rt)
                first_score = score_tile(0)
                add_dep_helper(
                    first_score.ins,
                    last_drain5.ins,
                    sync=False,
                    reason="drains 0-5 before any RS-gated score",
                )
                for rt in range(1, 6):
                    score_tile(rt)
                for rt in range(6, RT):
                    kv_tile(rt)
                for rt in range(6, RT):
                    score_tile(rt)

            if stage == "kv":
                nc.scalar.dma_start(out_d[:, 0 : 2 * RT], s_scr[0:AG, 0 : 2 * RT])

            # ---------------- softmax over t (16 steps per agent) ----------------

            if do_tail:
                # gather upper-half scores (odd steps) into cols RT..2*RT
                nc.scalar.dma_start(s_scr[0:AG, RT : 2 * RT], s_scr[AG:128, 0:RT])
                negmax = res.tile([AG, 1], f32)
                nc.vector.tensor_reduce(
                    negmax[:], s_scr[0:AG, :], mybir.AxisListType.X, mybir.AluOpType.max,
                    negate=True,
                )
                ex = res.tile([AG, 2 * RT], f32)
                sume = res.tile([AG, 1], f32)
                nc.scalar.activation(
                    ex[:],
                    s_scr[0:AG, :],
                    mybir.ActivationFunctionType.Exp,
                    bias=negmax[:],
                    accum_out=sume[:],
                )
                rcp = res.tile([AG, 1], f32)
                nc.vector.reciprocal(rcp[:], sume[:])
                al_n = res.tile([AG, 2 * RT], f32)
                nc.vector.tensor_scalar_mul(al_n[:], ex[:], rcp[:])

                # scatter normalized alpha back to the scrambled [128, RT] layout
                al = res.tile([128, RT], f32)
                nc.vector.tensor_copy(al[0:AG, :], al_n[:, 0:RT])
                nc.scalar.dma_start(al[AG:128, :], al_n[0:AG, RT : 2 * RT])

                # ---------------- weighted sum of v ----------------
                macc = res.tile([128, DV], f32)
                nc.vector.tensor_scalar_mul(macc[:], v_sb[:, 0, :], al[:, 0:1])
                for rt in range(1, RT):
                    nc.vector.scalar_tensor_tensor(
                        macc[:], v_sb[:, rt, :], al[:, rt : rt + 1], macc[:], mult, add
                    )

                mfin = res.tile([AG, DV], f32)
                nc.vector.tensor_tensor(
                    mfin[:], macc[0:AG, :], bv_sb[:], add
                )
                nc.scalar.dma_start(out_d[:], mfin[:])
                # add the odd-step half directly into DRAM (CCE accumulate)
                nc.gpsimd.dma_start(out_d[:], macc[AG:128, :], accum_op=add)

    nc.compile()
    return nc


# packed q column p = m*128 + 16*c + j  ->  original agent 64*c + 16*m + j
_AGENT_PERM = np.array(
    [
        64 * ((p % 128) // 16) + 16 * (p // 128) + (p % 16)
        for p in range(N_AGENTS)
    ],
    dtype=np.int64,
)


def _cdt():
    if DTYPE == "bf16":
        import ml_dtypes

        return ml_dtypes.bfloat16
    return np.float32


def _pack(a, kchunks, inner):
    # [K, N] -> [128, K//128, N] with the contraction dim on partitions
    return np.ascontiguousarray(
        a.reshape(kchunks, 128, inner).transpose(1, 0, 2), dtype=_cdt()
    )


def _make_in_maps(
    imagined_trajectory, received_messages, Wq, bq, Wk, bk, Wv, bv
):
    imagined_trajectory = np.asarray(imagined_trajectory, dtype=np.float32)
    received_messages = np.asarray(received_messages, dtype=np.float32)
    Wq = np.asarray(Wq, dtype=np.float32)
    bq = np.asarray(bq, dtype=np.float32)
    Wk = np.asarray(Wk, dtype=np.float32)
    Wv = np.asarray(Wv, dtype=np.float32)
    bv = np.asarray(bv, dtype=np.float32)

    wk_p = _pack(Wk, KT, HID)
    wv_p = _pack(Wv, KT, DV)
    bqs = np.ascontiguousarray(
        np.broadcast_to(bq * SCALE / NC, (128, HID)), dtype=np.float32
    )
    bv_r = np.ascontiguousarray(np.broadcast_to(bv, (AG, DV)), dtype=np.float32)

    in_maps = []
    for c in range(NC):
        g, s = c // GS, c % GS
        gslice = slice(g * GAG, (g + 1) * GAG)
        mslice = slice(s * MS, (s + 1) * MS)
        rm_t = received_messages[gslice, mslice].T  # [8192, 256]
        wq_sh = Wq[mslice, :] * SCALE  # [4096, 512], pre-scaled
        taus = imagined_trajectory[:, c * AG : (c + 1) * AG, :].reshape(T * AG, TAU)
        traj_t = taus.T  # [2048, 1024]
        rm_p = np.ascontiguousarray(
            rm_t.reshape(KQ, 128, GAG // 128, 128).transpose(2, 1, 0, 3),
            dtype=_cdt(),
        )
        traj_p = np.ascontiguousarray(
            traj_t.reshape(KT, 128, RT, 128).transpose(2, 1, 0, 3), dtype=_cdt()
        )
        in_maps.append(
            {
                "rm": rm_p,
                "wq": _pack(wq_sh, KQ, HID),
                "traj": traj_p,
                "wk": wk_p,
                "wv": wv_p,
                "bqs": bqs,
                "bv": bv_r,
            }
        )
    return in_maps


def kernel(
    imagined_trajectory,
    received_messages,
    Wq,
    bq,
    Wk,
    bk,
    Wv,
    bv,
):
    global LAST_EXEC_NS, LAST_RESULTS
    from concourse.bass_utils import run_bass_kernel_spmd

    if "nc" not in _CACHE:
        _CACHE["nc"] = _build()
    nc = _CACHE["nc"]

    in_maps = _make_in_maps(
        imagined_trajectory, received_messages, Wq, bq, Wk, bk, Wv, bv
    )

    res = run_bass_kernel_spmd(
        nc,
        in_maps,
        core_ids=list(range(NC)),
        trace=TRACE,
        trace_cores=None,
    )
    LAST_EXEC_NS = res.exec_time_ns
    LAST_RESULTS = res
    out = np.concatenate([res.results[c]["out"] for c in range(NC)], axis=0)
    return out.astype(np.float32)

